# revision 3
# baseline (speedup 1.0000x reference)
"""Causal single-head attention (B=16, T=2048, C=288, hs=32) on 8 TRN2 cores.

Reference (note the k/q swap -- weights = einsum("bth,bsh->bts", k, q)):
    k = x @ Wk; q = x @ Wq; v = x @ Wv
    S[t, s] = k[t] . q[s] / sqrt(hs), causal (s <= t), softmax over s
    out = softmax(S) @ v

Sharding: data-parallel over batch, 2 batches per core, no collectives.

v2 design (ACT-bound kernel; everything else overlaps under the exps):
  - Pipelined per 512-column t-group: load x slice -> project -> attend,
    with the projection stream running one group ahead of attention.
  - kT/qT are produced 4x-replicated across partition groups (weights
    pre-replicated [C,128] on host) so the K=32 score matmuls can be
    row-packed 4x via tile_position -- 4 concurrent 32x128 PE tiles.
  - Scores for 4 s-chunks land in 2-bank PSUM pairs; exp runs as one
    [128, 1024] ACT instruction per pair (fixed ACT overhead halved).
  - Softmax denominator via ones-column appended to V (PV matmul
    computes numerator and denominator together).
  - All four PV accumulators of a t-group share ONE PSUM bank; V
    projection accumulates all 16 row-blocks into one bank (start=True
    only on the first matmul into the bank -- untouched elements are
    overwritten because their has_written bits are clear).
  - x loads issued from the SP queue, output stores from the GpSimd
    queue, keeping DMA issue off the critical engines.

Softmax is computed without max-subtraction: scores are ~N(0,1) by
construction, so exp never overflows in fp32.
"""

import ml_dtypes
import numpy as np

import concourse.bass as bass
import concourse.mybir as mybir
from concourse.tile import TileContext
from concourse.bass_utils import run_bass_kernel_spmd

# ---------------------------------------------------------------- constants
B, T, C, HS = 16, 2048, 288, 32
N_CORES = 8
BPC = B // N_CORES          # batches per core
P = 128                     # partition block
TG = 512                    # t-columns per group
NT = T // P                 # 16 s-chunks / t-row-blocks
NG = T // TG                # 4 t-groups
CCHUNKS = [(0, 128), (128, 128), (256, 32)]   # C=288 split for partitions
SCALE = float(HS) ** -0.5
VW = HS + 1                 # V1 block width (ones column appended)

COMPUTE_DT = mybir.dt.bfloat16      # matmul operand dtype
E_DT = mybir.dt.bfloat16            # dtype of exp(S) fed to the PV matmul
NP_COMPUTE_DT = np.dtype(ml_dtypes.bfloat16)

# Schraudolph fast-exp on DVE: bf16(exp(x)) ~= bitcast_i16(round(A*x + B)).
# C=7 zeroes the mean log-ratio so the ACT (exact) and DVE (approx) exp
# paths agree in expectation and softmax error cancellation survives mixing.
import math
SCH_A = (2.0 ** 7 / math.log(2.0)) * SCALE
SCH_B = float(127 * 2 ** 7 - 7)
# fraction of off-diagonal score pairs whose exp runs on DVE instead of ACT
DVE_EXP_PATTERN = (True, True, False, False, False)


def _split_multi_waits(nc: bass.Bass) -> int:
    """This walrus build accepts only ONE sync-wait command per instruction
    (setupSyncWait<...> raises "Too many sync wait commands" otherwise), but
    Tile's semaphore assignment attaches one wait per depended-on processor.
    Move all but the last wait of each instruction onto dedicated same-engine
    NOPs placed immediately before it -- the engine stalls at the NOPs first,
    so ordering semantics are identical."""
    cnt = 0
    for f in nc.m.functions:
        for bb in f.blocks:
            new_insts = []
            for inst in bb.instructions:
                si = getattr(inst, "sync_info", None)
                if si is not None and si.on_wait and len(si.on_wait) > 1:
                    extra = list(si.on_wait[:-1])
                    del si.on_wait[:-1]
                    for w in extra:
                        cnt += 1
                        new_insts.append(
                            mybir.InstNoOp(
                                name=f"{inst.name}-wsplit{cnt}",
                                sync_info=mybir.SyncInfo(on_wait=[w], on_update=[]),
                                bass_nofuse=True,
                                engine=inst.engine,
                            )
                        )
                new_insts.append(inst)
            bb.instructions[:] = new_insts
    return cnt


def build_attention_nc(reps: int = 1) -> bass.Bass:
    nc = bass.Bass()
    cdt = COMPUTE_DT

    xt = nc.dram_tensor("xt", [BPC, C, T], cdt, kind="ExternalInput")
    # [Wk x4 | Wq x4 | Wv] pre-replicated on host -> [C, 288]
    wall = nc.dram_tensor("wall", [C, 2 * P + HS], cdt, kind="ExternalInput")
    tri = nc.dram_tensor("tri", [P, P], E_DT, kind="ExternalInput")
    out = nc.dram_tensor("out", [BPC, T, HS], mybir.dt.float32, kind="ExternalOutput")

    with TileContext(nc) as tc:
        with (
            tc.tile_pool(name="consts", bufs=1) as cpool,
            tc.tile_pool(name="xg", bufs=3) as xg_pool,
            tc.tile_pool(name="kqv", bufs=2) as kqv_pool,
            tc.tile_pool(name="e", bufs=14) as e_pool,
            tc.tile_pool(name="outp", bufs=4) as out_pool,
            tc.tile_pool(name="pair", bufs=3, space="PSUM") as pair_pool,
            tc.tile_pool(name="acc", bufs=1, space="PSUM") as acc_pool,
        ):
            # ---- constants
            tri_sb = cpool.tile([P, P], E_DT, tag="tri")
            nc.sync.dma_start(tri_sb[:], tri[:, :])
            wk_sb, wq_sb, wv_sb = [], [], []
            for ci, (coff, csz) in enumerate(CCHUNKS):
                wt = cpool.tile([csz, 2 * P + HS], cdt, tag=f"w{ci}", name=f"w{ci}")
                nc.sync.dma_start(wt[:], wall[coff : coff + csz, :])
                wk_sb.append(wt[:, 0:P])
                wq_sb.append(wt[:, P : 2 * P])
                wv_sb.append(wt[:, 2 * P : 2 * P + HS])

            def body():
                state = {}
                ecnt = [0]

                def pair_exp(dst_bf16_of, psrc, epool_tag):
                    """exp of one [128, 2*TG] psum pair -> bf16 SBUF tile.
                    Returns the bf16-viewed tile."""
                    use_dve = DVE_EXP_PATTERN[ecnt[0] % len(DVE_EXP_PATTERN)]
                    ecnt[0] += 1
                    if use_dve:
                        ei = e_pool.tile([P, 2 * TG], mybir.dt.int16, tag="ei")
                        nc.vector.tensor_scalar(
                            ei[:], psrc[:], SCH_A, SCH_B,
                            op0=mybir.AluOpType.mult, op1=mybir.AluOpType.add,
                        )
                        return ei.bitcast(E_DT)
                    e = e_pool.tile([P, 2 * TG], E_DT, tag="e")
                    nc.scalar.activation(
                        e[:], psrc[:], mybir.ActivationFunctionType.Exp,
                        scale=SCALE,
                    )
                    return e

                def proj(b, g):
                    """Load x slice (b, g); project kqT group g, V blocks
                    4g..4g+3."""
                    if g == 0:
                        st = state[b] = {}
                        # kq: group g at cols [2*TG*g, 2*TG*(g+1)) --
                        # kT in the first 512, qT in the second 512; each
                        # 32-partition group holds a full replica.
                        st["kq"] = kqv_pool.tile(
                            [P, 2 * T], cdt, tag="kq", name=f"kq_{b}"
                        )
                        st["v1"] = kqv_pool.tile(
                            [P, NT, VW], cdt, tag="v1", name=f"v1_{b}"
                        )
                        nc.vector.memset(st["v1"][:, :, HS : HS + 1], 1.0)
                        st["vps"] = acc_pool.tile(
                            [P, NT * HS], mybir.dt.float32, tag="vps",
                            name=f"vps_{b}",
                        )
                        st["e"] = {}
                    st = state[b]
                    xg = []
                    for ci, (coff, csz) in enumerate(CCHUNKS):
                        t_ = xg_pool.tile([csz, TG], cdt, tag=f"x{ci}")
                        nc.sync.dma_start(
                            t_[:], xt[b, coff : coff + csz, g * TG : (g + 1) * TG]
                        )
                        xg.append(t_)
                    # kq projection -> one 2-bank psum pair, one SBUF copy
                    pk = pair_pool.tile([P, 2 * TG], mybir.dt.float32, tag="pair")
                    for ci in range(3):
                        nc.tensor.matmul(
                            pk[:, 0:TG],
                            lhsT=wk_sb[ci],
                            rhs=xg[ci][:],
                            start=(ci == 0),
                            stop=(ci == 2),
                        )
                    for ci in range(3):
                        nc.tensor.matmul(
                            pk[:, TG : 2 * TG],
                            lhsT=wq_sb[ci],
                            rhs=xg[ci][:],
                            start=(ci == 0),
                            stop=(ci == 2),
                        )
                    nc.vector.tensor_copy(
                        st["kq"][:, g * 2 * TG : (g + 1) * 2 * TG], pk[:]
                    )
                    # V blocks 4g..4g+3 -> one shared psum bank (cols 128g..)
                    for tt in range(4):
                        for ci in range(3):
                            nc.tensor.matmul(
                                st["vps"][
                                    :, (4 * g + tt) * HS : (4 * g + tt + 1) * HS
                                ],
                                lhsT=xg[ci][:, tt * P : (tt + 1) * P],
                                rhs=wv_sb[ci],
                                start=(g == 0 and tt == 0 and ci == 0),
                                stop=(ci == 2),
                                skip_group_check=True,
                            )
                    nc.vector.tensor_copy(
                        st["v1"][:, 4 * g : 4 * g + 4, 0:HS],
                        st["vps"][:, 4 * g * HS : (4 * g + 4) * HS],
                    )

                def attn_scores(b, g):
                    """Score matmuls + exp + causal mask for group g."""
                    st = state[b]
                    emap = st["e"][g] = {}
                    kq = st["kq"]
                    t0 = g * TG

                    def qslice(l, j):
                        gs, m = j // 4, j % 4
                        base = gs * 2 * TG + TG + m * P
                        return kq[32 * l : 32 * l + 32, base : base + P]

                    # off-diagonal quads: 4x row-packed, full width
                    for q in range(g):
                        pa = pair_pool.tile([P, 2 * TG], mybir.dt.float32, tag="pair")
                        pb = pair_pool.tile([P, 2 * TG], mybir.dt.float32, tag="pair")
                        for l in range(4):
                            dst = (pa if l < 2 else pb)[
                                :, (l % 2) * TG : (l % 2 + 1) * TG
                            ]
                            nc.tensor.matmul(
                                dst,
                                lhsT=qslice(l, 4 * q + l),
                                rhs=kq[
                                    32 * l : 32 * l + 32,
                                    g * 2 * TG : g * 2 * TG + TG,
                                ],
                                start=True,
                                stop=True,
                                tile_position=(32 * l, 0),
                            )
                        ea = pair_exp(None, pa, "e")
                        eb = pair_exp(None, pb, "e")
                        for l in range(4):
                            emap[4 * q + l] = (
                                (ea if l < 2 else eb), (l % 2) * TG, TG,
                            )
                    # diagonal quad: serial on row-tile 0, tapering widths
                    pa = pair_pool.tile([P, 2 * TG], mybir.dt.float32, tag="pair")
                    pc = pair_pool.tile([P, 2 * TG], mybir.dt.float32, tag="pair")
                    offs = [(pa, 0, TG), (pa, TG, 384), (pc, 0, 256), (pc, 256, P)]
                    for l in range(4):
                        j = 4 * g + l
                        dstt, doff, width = offs[l]
                        nc.tensor.matmul(
                            dstt[:, doff : doff + width],
                            lhsT=qslice(0, j),
                            rhs=kq[
                                0:32,
                                g * 2 * TG + l * P : g * 2 * TG + TG,
                            ],
                            start=True,
                            stop=True,
                            tile_position=(0, 0),
                        )
                    ea = e_pool.tile([P, 2 * TG], E_DT, tag="e")
                    ec = e_pool.tile([P, TG], E_DT, tag="ec", bufs=3)
                    nc.scalar.activation(
                        ea[:, 0 : TG + 384], pa[:, 0 : TG + 384],
                        mybir.ActivationFunctionType.Exp, scale=SCALE,
                    )
                    nc.scalar.activation(
                        ec[:, 0:384], pc[:, 0:384],
                        mybir.ActivationFunctionType.Exp, scale=SCALE,
                    )
                    emap[4 * g + 0] = (ea, 0, TG)
                    emap[4 * g + 1] = (ea, TG, 384)
                    emap[4 * g + 2] = (ec, 0, 256)
                    emap[4 * g + 3] = (ec, 256, P)
                    # causal mask on the four diagonal 128-blocks
                    for l in range(4):
                        et, off, width = emap[4 * g + l]
                        nc.gpsimd.tensor_mul(
                            et[:, off : off + P], et[:, off : off + P], tri_sb[:]
                        )

                def attn_pv(b, g):
                    """PV accumulation, normalize, store for group g."""
                    st = state[b]
                    emap = st["e"].pop(g)
                    po = acc_pool.tile([P, 4, VW], mybir.dt.float32, tag="po", bufs=1)
                    for j in range(4 * g + 4):
                        et, off, width = emap[j]
                        cstart = max(g * TG, j * P)
                        for i in range(max(4 * g, j), 4 * g + 4):
                            ii = i - 4 * g
                            eoff = off + (i * P - cstart)
                            nc.tensor.matmul(
                                po[:, ii, :],
                                lhsT=et[:, eoff : eoff + P],
                                rhs=st["v1"][:, j, :],
                                start=(j == 0 and ii == 0),
                                stop=(j == i),
                                skip_group_check=True,
                            )
                    rec = out_pool.tile([P, 4], mybir.dt.float32, tag="rec")
                    nc.vector.reciprocal(rec[:], po[:, :, HS : HS + 1])
                    ot = out_pool.tile([P, 4, HS], mybir.dt.float32, tag="ot")
                    for ii in range(4):
                        nc.vector.tensor_scalar_mul(
                            ot[:, ii, :], po[:, ii, 0:HS], rec[:, ii : ii + 1]
                        )
                    dst = out[b, g * TG : (g + 1) * TG, :].rearrange(
                        "(i p) h -> p i h", p=P
                    )
                    nc.sync.dma_start(dst, ot[:])

                # pipeline: PROJ stream one group ahead of the ATTN stream
                steps = [(b, g) for b in range(BPC) for g in range(NG)]
                # PV runs one step behind scores, and the projection stream
                # two steps ahead, so the next group's score matmuls are
                # already runnable on PE when ACT drains its exps.
                proj(*steps[0])
                proj(*steps[1])
                for idx, (b, g) in enumerate(steps):
                    attn_scores(b, g)
                    if idx > 0:
                        attn_pv(*steps[idx - 1])
                    if idx + 2 < len(steps):
                        proj(*steps[idx + 2])
                attn_pv(*steps[-1])

            if reps == 1:
                body()
            else:
                with tc.For_i(
                    0,
                    reps,
                    1,
                    hint_engines=(
                        mybir.EngineType.PE,
                        mybir.EngineType.Activation,
                        mybir.EngineType.DVE,
                        mybir.EngineType.SP,
                        mybir.EngineType.Pool,
                    ),
                ):
                    body()
    _split_multi_waits(nc)
    return nc


_NC_CACHE: dict = {}


def _get_nc(reps: int = 1) -> bass.Bass:
    if reps not in _NC_CACHE:
        _NC_CACHE[reps] = build_attention_nc(reps)
    return _NC_CACHE[reps]


def make_in_maps(x, Wk, Wq, Wv):
    x = np.asarray(x, dtype=np.float32)
    xt = np.ascontiguousarray(x.transpose(0, 2, 1)).astype(NP_COMPUTE_DT)
    wk = np.asarray(Wk, dtype=np.float32).astype(NP_COMPUTE_DT)
    wq = np.asarray(Wq, dtype=np.float32).astype(NP_COMPUTE_DT)
    wv = np.asarray(Wv, dtype=np.float32).astype(NP_COMPUTE_DT)
    wall = np.ascontiguousarray(
        np.concatenate([np.tile(wk, (1, 4)), np.tile(wq, (1, 4)), wv], axis=1)
    )
    tri = np.triu(np.ones((P, P), dtype=np.float32)).astype(
        np.dtype(ml_dtypes.bfloat16)
    )
    in_maps = []
    for c in range(N_CORES):
        in_maps.append(
            {
                "xt": np.ascontiguousarray(xt[c * BPC : (c + 1) * BPC]),
                "wall": wall,
                "tri": tri,
            }
        )
    return in_maps


def kernel(x, Wk, Wq, Wv) -> np.ndarray:
    nc = _get_nc(reps=1)
    in_maps = make_in_maps(x, Wk, Wq, Wv)
    res = run_bass_kernel_spmd(nc, in_maps, core_ids=list(range(N_CORES)))
    return np.concatenate([r["out"] for r in res.results], axis=0)


# revision 4
# speedup vs baseline: 1.0554x; 1.0554x over previous
"""Causal single-head attention (B=16, T=2048, C=288, hs=32) on 8 TRN2 cores.

Reference (note the k/q swap -- weights = einsum("bth,bsh->bts", k, q)):
    k = x @ Wk; q = x @ Wq; v = x @ Wv
    S[t, s] = k[t] . q[s] / sqrt(hs), causal (s <= t), softmax over s
    out = softmax(S) @ v

Sharding: data-parallel over batch, 2 batches per core, no collectives.

v2 design (ACT-bound kernel; everything else overlaps under the exps):
  - Pipelined per 512-column t-group: load x slice -> project -> attend,
    with the projection stream running one group ahead of attention.
  - kT/qT are produced 4x-replicated across partition groups (weights
    pre-replicated [C,128] on host) so the K=32 score matmuls can be
    row-packed 4x via tile_position -- 4 concurrent 32x128 PE tiles.
  - Scores for 4 s-chunks land in 2-bank PSUM pairs; exp runs as one
    [128, 1024] ACT instruction per pair (fixed ACT overhead halved).
  - Softmax denominator via ones-column appended to V (PV matmul
    computes numerator and denominator together).
  - All four PV accumulators of a t-group share ONE PSUM bank; V
    projection accumulates all 16 row-blocks into one bank (start=True
    only on the first matmul into the bank -- untouched elements are
    overwritten because their has_written bits are clear).
  - x loads issued from the SP queue, output stores from the GpSimd
    queue, keeping DMA issue off the critical engines.

Softmax is computed without max-subtraction: scores are ~N(0,1) by
construction, so exp never overflows in fp32.
"""

import ml_dtypes
import numpy as np

import concourse.bass as bass
import concourse.mybir as mybir
from concourse.tile import TileContext
from concourse.bass_utils import run_bass_kernel_spmd

# ---------------------------------------------------------------- constants
B, T, C, HS = 16, 2048, 288, 32
N_CORES = 8
BPC = B // N_CORES          # batches per core
P = 128                     # partition block
TG = 512                    # t-columns per group
NT = T // P                 # 16 s-chunks / t-row-blocks
NG = T // TG                # 4 t-groups
CCHUNKS = [(0, 128), (128, 128), (256, 32)]   # C=288 split for partitions
SCALE = float(HS) ** -0.5
VW = HS + 1                 # V1 block width (ones column appended)

COMPUTE_DT = mybir.dt.bfloat16      # matmul operand dtype
E_DT = mybir.dt.bfloat16            # dtype of exp(S) fed to the PV matmul
NP_COMPUTE_DT = np.dtype(ml_dtypes.bfloat16)

# Schraudolph fast-exp on DVE: bf16(exp(x)) ~= bitcast_i16(round(A*x + B)).
# C=7 zeroes the mean log-ratio so the ACT (exact) and DVE (approx) exp
# paths agree in expectation and softmax error cancellation survives mixing.
import math
SCH_A = (2.0 ** 7 / math.log(2.0)) * SCALE
SCH_B = float(127 * 2 ** 7 - 7)
# fraction of off-diagonal score pairs whose exp runs on DVE instead of ACT
DVE_EXP_PATTERN = (True, False)
# which kq PSUM->SBUF copies run as ACT Copy-activations instead of DVE
KQ_COPY_ACT_PATTERN = (True,)


def _split_multi_waits(nc: bass.Bass) -> int:
    """This walrus build accepts only ONE sync-wait command per instruction
    (setupSyncWait<...> raises "Too many sync wait commands" otherwise), but
    Tile's semaphore assignment attaches one wait per depended-on processor.
    Move all but the last wait of each instruction onto dedicated same-engine
    NOPs placed immediately before it -- the engine stalls at the NOPs first,
    so ordering semantics are identical."""
    cnt = 0
    for f in nc.m.functions:
        for bb in f.blocks:
            new_insts = []
            for inst in bb.instructions:
                si = getattr(inst, "sync_info", None)
                if si is not None and si.on_wait and len(si.on_wait) > 1:
                    extra = list(si.on_wait[:-1])
                    del si.on_wait[:-1]
                    for w in extra:
                        cnt += 1
                        new_insts.append(
                            mybir.InstNoOp(
                                name=f"{inst.name}-wsplit{cnt}",
                                sync_info=mybir.SyncInfo(on_wait=[w], on_update=[]),
                                bass_nofuse=True,
                                engine=inst.engine,
                            )
                        )
                new_insts.append(inst)
            bb.instructions[:] = new_insts
    return cnt


def build_attention_nc(reps: int = 1) -> bass.Bass:
    nc = bass.Bass()
    cdt = COMPUTE_DT

    xt = nc.dram_tensor("xt", [BPC, C, T], cdt, kind="ExternalInput")
    # [Wk x4 | Wq x4 | Wv] pre-replicated on host -> [C, 288]
    wall = nc.dram_tensor("wall", [C, 2 * P + HS], cdt, kind="ExternalInput")
    tri = nc.dram_tensor("tri", [P, P], E_DT, kind="ExternalInput")
    out = nc.dram_tensor("out", [BPC, T, HS], mybir.dt.float32, kind="ExternalOutput")

    with TileContext(nc) as tc:
        with (
            tc.tile_pool(name="consts", bufs=1) as cpool,
            tc.tile_pool(name="xg", bufs=3) as xg_pool,
            tc.tile_pool(name="kqv", bufs=2) as kqv_pool,
            tc.tile_pool(name="e", bufs=14) as e_pool,
            tc.tile_pool(name="outp", bufs=4) as out_pool,
            tc.tile_pool(name="pair", bufs=3, space="PSUM") as pair_pool,
            tc.tile_pool(name="acc", bufs=1, space="PSUM") as acc_pool,
        ):
            # ---- constants
            tri_sb = cpool.tile([P, P], E_DT, tag="tri")
            nc.sync.dma_start(tri_sb[:], tri[:, :])
            wk_sb, wq_sb, wv_sb = [], [], []
            for ci, (coff, csz) in enumerate(CCHUNKS):
                wt = cpool.tile([csz, 2 * P + HS], cdt, tag=f"w{ci}", name=f"w{ci}")
                nc.sync.dma_start(wt[:], wall[coff : coff + csz, :])
                wk_sb.append(wt[:, 0:P])
                wq_sb.append(wt[:, P : 2 * P])
                wv_sb.append(wt[:, 2 * P : 2 * P + HS])

            def body():
                state = {}
                ecnt = [0]
                kqcnt = [0]

                def kq_copy(dst, src):
                    use_act = KQ_COPY_ACT_PATTERN[
                        kqcnt[0] % len(KQ_COPY_ACT_PATTERN)
                    ]
                    kqcnt[0] += 1
                    if use_act:
                        nc.scalar.activation(
                            dst, src, mybir.ActivationFunctionType.Copy
                        )
                    else:
                        nc.vector.tensor_copy(dst, src)

                def pair_exp(dst_bf16_of, psrc, epool_tag, width=2 * TG):
                    """exp of one psum pair region -> bf16 SBUF tile."""
                    use_dve = DVE_EXP_PATTERN[ecnt[0] % len(DVE_EXP_PATTERN)]
                    ecnt[0] += 1
                    if use_dve:
                        ei = e_pool.tile(
                            [P, 2 * TG], mybir.dt.int16, tag="ei" + epool_tag
                        )
                        nc.vector.tensor_scalar(
                            ei[:, 0:width], psrc[:, 0:width], SCH_A, SCH_B,
                            op0=mybir.AluOpType.mult, op1=mybir.AluOpType.add,
                        )
                        return ei.bitcast(E_DT)
                    e = e_pool.tile([P, 2 * TG], E_DT, tag="e" + epool_tag)
                    nc.scalar.activation(
                        e[:, 0:width], psrc[:, 0:width],
                        mybir.ActivationFunctionType.Exp, scale=SCALE,
                    )
                    return e

                def proj(b, g):
                    """Load x slice (b, g); project kqT group g, V blocks
                    4g..4g+3."""
                    if g == 0:
                        st = state[b] = {}
                        # kq: group g at cols [2*TG*g, 2*TG*(g+1)) --
                        # kT in the first 512, qT in the second 512; each
                        # 32-partition group holds a full replica.
                        st["kq"] = kqv_pool.tile(
                            [P, 2 * T], cdt, tag="kq", name=f"kq_{b}"
                        )
                        st["v1"] = kqv_pool.tile(
                            [P, NT, VW], cdt, tag="v1", name=f"v1_{b}"
                        )
                        nc.vector.memset(st["v1"][:, :, HS : HS + 1], 1.0)
                        st["vps"] = acc_pool.tile(
                            [P, NT * HS], mybir.dt.float32, tag="vps",
                            name=f"vps_{b}",
                        )
                        st["e"] = {}
                    st = state[b]
                    xg = []
                    for ci, (coff, csz) in enumerate(CCHUNKS):
                        t_ = xg_pool.tile([csz, TG], cdt, tag=f"x{ci}")
                        nc.sync.dma_start(
                            t_[:], xt[b, coff : coff + csz, g * TG : (g + 1) * TG]
                        )
                        xg.append(t_)
                    # kq projection -> one 2-bank psum pair, one SBUF copy
                    pk = pair_pool.tile([P, 2 * TG], mybir.dt.float32, tag="pair")
                    for ci in range(3):
                        nc.tensor.matmul(
                            pk[:, 0:TG],
                            lhsT=wk_sb[ci],
                            rhs=xg[ci][:],
                            start=(ci == 0),
                            stop=(ci == 2),
                        )
                    for ci in range(3):
                        nc.tensor.matmul(
                            pk[:, TG : 2 * TG],
                            lhsT=wq_sb[ci],
                            rhs=xg[ci][:],
                            start=(ci == 0),
                            stop=(ci == 2),
                        )
                    kq_copy(st["kq"][:, g * 2 * TG : (g + 1) * 2 * TG], pk[:])
                    # V blocks 4g..4g+3 -> one shared psum bank (cols 128g..)
                    for tt in range(4):
                        for ci in range(3):
                            nc.tensor.matmul(
                                st["vps"][
                                    :, (4 * g + tt) * HS : (4 * g + tt + 1) * HS
                                ],
                                lhsT=xg[ci][:, tt * P : (tt + 1) * P],
                                rhs=wv_sb[ci],
                                start=(g == 0 and tt == 0 and ci == 0),
                                stop=(ci == 2),
                                skip_group_check=True,
                            )
                    nc.vector.tensor_copy(
                        st["v1"][:, 4 * g : 4 * g + 4, 0:HS],
                        st["vps"][:, 4 * g * HS : (4 * g + 4) * HS],
                    )

                def attn_scores(b, g):
                    """Score matmuls + exp + causal mask for group g."""
                    st = state[b]
                    emap = st["e"][g] = {}
                    kq = st["kq"]
                    t0 = g * TG

                    def qslice(l, j):
                        gs, m = j // 4, j % 4
                        base = gs * 2 * TG + TG + m * P
                        return kq[32 * l : 32 * l + 32, base : base + P]

                    # off-diagonal quads: 4x row-packed, full width
                    for q in range(g):
                        pa = pair_pool.tile([P, 2 * TG], mybir.dt.float32, tag="pair")
                        pb = pair_pool.tile([P, 2 * TG], mybir.dt.float32, tag="pair")
                        for l in range(4):
                            dst = (pa if l < 2 else pb)[
                                :, (l % 2) * TG : (l % 2 + 1) * TG
                            ]
                            nc.tensor.matmul(
                                dst,
                                lhsT=qslice(l, 4 * q + l),
                                rhs=kq[
                                    32 * l : 32 * l + 32,
                                    g * 2 * TG : g * 2 * TG + TG,
                                ],
                                start=True,
                                stop=True,
                                tile_position=(32 * l, 0),
                            )
                        ea = pair_exp(None, pa, "")
                        eb = pair_exp(None, pb, "")
                        for l in range(4):
                            emap[4 * q + l] = (
                                (ea if l < 2 else eb), (l % 2) * TG, TG,
                            )
                    # diagonal quad: serial on row-tile 0, tapering widths
                    pa = pair_pool.tile([P, 2 * TG], mybir.dt.float32, tag="pair")
                    pc = pair_pool.tile([P, 2 * TG], mybir.dt.float32, tag="pair")
                    offs = [(pa, 0, TG), (pa, TG, 384), (pc, 0, 256), (pc, 256, P)]
                    for l in range(4):
                        j = 4 * g + l
                        dstt, doff, width = offs[l]
                        nc.tensor.matmul(
                            dstt[:, doff : doff + width],
                            lhsT=qslice(0, j),
                            rhs=kq[
                                0:32,
                                g * 2 * TG + l * P : g * 2 * TG + TG,
                            ],
                            start=True,
                            stop=True,
                            tile_position=(0, 0),
                        )
                    ea = e_pool.tile([P, 2 * TG], E_DT, tag="e")
                    ec = e_pool.tile([P, TG], E_DT, tag="ec", bufs=3)
                    nc.scalar.activation(
                        ea[:, 0 : TG + 384], pa[:, 0 : TG + 384],
                        mybir.ActivationFunctionType.Exp, scale=SCALE,
                    )
                    nc.scalar.activation(
                        ec[:, 0:384], pc[:, 0:384],
                        mybir.ActivationFunctionType.Exp, scale=SCALE,
                    )
                    emap[4 * g + 0] = (ea, 0, TG)
                    emap[4 * g + 1] = (ea, TG, 384)
                    emap[4 * g + 2] = (ec, 0, 256)
                    emap[4 * g + 3] = (ec, 256, P)
                    # causal mask on the four diagonal 128-blocks
                    for l in range(4):
                        et, off, width = emap[4 * g + l]
                        nc.gpsimd.tensor_mul(
                            et[:, off : off + P], et[:, off : off + P], tri_sb[:]
                        )

                def attn_pv(b, g):
                    """PV accumulation, normalize, store for group g."""
                    st = state[b]
                    emap = st["e"].pop(g)
                    po = acc_pool.tile([P, 4, VW], mybir.dt.float32, tag="po", bufs=1)
                    for j in range(4 * g + 4):
                        et, off, width = emap[j]
                        cstart = max(g * TG, j * P)
                        for i in range(max(4 * g, j), 4 * g + 4):
                            ii = i - 4 * g
                            eoff = off + (i * P - cstart)
                            nc.tensor.matmul(
                                po[:, ii, :],
                                lhsT=et[:, eoff : eoff + P],
                                rhs=st["v1"][:, j, :],
                                start=(j == 0 and ii == 0),
                                stop=(j == i),
                                skip_group_check=True,
                            )
                    rec = out_pool.tile([P, 4], mybir.dt.float32, tag="rec")
                    nc.vector.reciprocal(rec[:], po[:, :, HS : HS + 1])
                    ot = out_pool.tile([P, 4, HS], mybir.dt.float32, tag="ot")
                    for ii in range(4):
                        nc.vector.tensor_scalar_mul(
                            ot[:, ii, :], po[:, ii, 0:HS], rec[:, ii : ii + 1]
                        )
                    dst = out[b, g * TG : (g + 1) * TG, :].rearrange(
                        "(i p) h -> p i h", p=P
                    )
                    nc.sync.dma_start(dst, ot[:])

                # pipeline: PROJ stream one group ahead of the ATTN stream
                steps = [(b, g) for b in range(BPC) for g in range(NG)]
                # PV runs one step behind scores, and the projection stream
                # two steps ahead, so the next group's score matmuls are
                # already runnable on PE when ACT drains its exps.
                proj(*steps[0])
                proj(*steps[1])
                for idx, (b, g) in enumerate(steps):
                    attn_scores(b, g)
                    if idx > 0:
                        attn_pv(*steps[idx - 1])
                    if idx + 2 < len(steps):
                        proj(*steps[idx + 2])
                attn_pv(*steps[-1])

            if reps == 1:
                body()
            else:
                with tc.For_i(
                    0,
                    reps,
                    1,
                    hint_engines=(
                        mybir.EngineType.PE,
                        mybir.EngineType.Activation,
                        mybir.EngineType.DVE,
                        mybir.EngineType.SP,
                        mybir.EngineType.Pool,
                    ),
                ):
                    body()
    _split_multi_waits(nc)
    return nc


_NC_CACHE: dict = {}


def _get_nc(reps: int = 1) -> bass.Bass:
    if reps not in _NC_CACHE:
        _NC_CACHE[reps] = build_attention_nc(reps)
    return _NC_CACHE[reps]


def make_in_maps(x, Wk, Wq, Wv):
    x = np.asarray(x, dtype=np.float32)
    xt = np.ascontiguousarray(x.transpose(0, 2, 1)).astype(NP_COMPUTE_DT)
    wk = np.asarray(Wk, dtype=np.float32).astype(NP_COMPUTE_DT)
    wq = np.asarray(Wq, dtype=np.float32).astype(NP_COMPUTE_DT)
    wv = np.asarray(Wv, dtype=np.float32).astype(NP_COMPUTE_DT)
    wall = np.ascontiguousarray(
        np.concatenate([np.tile(wk, (1, 4)), np.tile(wq, (1, 4)), wv], axis=1)
    )
    tri = np.triu(np.ones((P, P), dtype=np.float32)).astype(
        np.dtype(ml_dtypes.bfloat16)
    )
    in_maps = []
    for c in range(N_CORES):
        in_maps.append(
            {
                "xt": np.ascontiguousarray(xt[c * BPC : (c + 1) * BPC]),
                "wall": wall,
                "tri": tri,
            }
        )
    return in_maps


def kernel(x, Wk, Wq, Wv) -> np.ndarray:
    nc = _get_nc(reps=1)
    in_maps = make_in_maps(x, Wk, Wq, Wv)
    res = run_bass_kernel_spmd(nc, in_maps, core_ids=list(range(N_CORES)))
    return np.concatenate([r["out"] for r in res.results], axis=0)
